# revision 23
# baseline (speedup 1.0000x reference)
"""DGCNN-EBLG forward on 8 TRN2 NeuronCores — batch data-parallel (2 samples/core).

Self-contained: hardcodes shapes B=16, C0=3, N=1024, K=20, 8 cores.

Per edge-conv layer (fp32):
  A  = W[:, :C] @ x ; Bc = (W[:, C:] - W[:, :C]) @ x
  pd = 2 x^T x - xx_i - xx_j ; idx = top20(pd)  (DVE max8, 3 rounds)
  g  = A[:, idx] (gpsimd ap_gather); Gm = max_k g ; G1 = sum_k g
  sum  = sum_n G1 + K*sum Bc
  ssq  = sum g^2 + 2 sum_n Bc*G1 + K*sum Bc^2
  AllReduce(sum, ssq) -> BN s,t ;  x' = leaky(s*(Gm + Bc) + t)  [max commutes]

O=64 layers pack both samples into one 128-partition block (s0 rows 0-63,
s1 rows 64-127). O=128: one block per sample. O=256: two blocks per sample.
"""
import numpy as np
import os

import concourse.bacc as bacc
import concourse.mybir as mybir
import concourse.tile as tile
import concourse.tile_utils as tile_utils
from concourse.tile import add_dep_helper
from concourse import library_config
from concourse.bass_utils import run_bass_kernel_spmd

tile_utils.max_sbuf_usage = 206 * 1024  # stale 192K default leaves SBUF unused

F32 = mybir.dt.float32
U16 = mybir.dt.uint16
I16 = mybir.dt.int16
AF = mybir.ActivationFunctionType
ALU = mybir.AluOpType
AX = mybir.AxisListType

NCORES = 8
B, C0, N, K = 16, 3, 1024, 20
BL = B // NCORES
NEG = -3.0e38
EPS = 1e-5

EDGE = [(3, 64), (64, 64), (64, 128), (128, 256)]
CNT_EDGE = float(B * N * K)
CNT_CONV = float(B * N)

_CACHE = {}


def prep_params(p):
    q = {}
    for li, (Wn, (C, O)) in enumerate(zip(["W1", "W4", "W7", "W10"], EDGE)):
        W = np.asarray(p[Wn], np.float32)
        q[f"wlT{li}"] = np.ascontiguousarray(W[:, :C].T)
        q[f"wdT{li}"] = np.ascontiguousarray((W[:, C:] - W[:, :C]).T)
    for li, (Wa_n, Wb_n) in enumerate(
        zip(["W2", "W5", "W8", "W11"], ["W3", "W6", "W9", "W12"])
    ):
        C = EDGE[li][1]
        Wa = np.asarray(p[Wa_n], np.float32)
        Wb = np.asarray(p[Wb_n], np.float32)
        q[f"walT{li}"] = np.ascontiguousarray(Wa[:, :C].T)
        q[f"warT{li}"] = np.ascontiguousarray(Wa[:, C:].T)
        q[f"wbT{li}"] = np.ascontiguousarray(Wb.T)
    q["w13T"] = np.ascontiguousarray(np.asarray(p["W13"], np.float32).T)
    q["l1T"] = np.ascontiguousarray(np.asarray(p["L1"], np.float32).T)
    q["l2T"] = np.ascontiguousarray(np.asarray(p["L2"], np.float32).T)
    q["l3T"] = np.ascontiguousarray(np.asarray(p["L3"], np.float32).T)
    for i in range(1, 16):
        q[f"g{i}"] = np.asarray(p[f"g{i}"], np.float32).reshape(-1, 1)
        q[f"b{i}"] = np.asarray(p[f"b{i}"], np.float32).reshape(-1, 1)
    q["bL2"] = np.asarray(p["bL2"], np.float32).reshape(-1, 1)
    q["bL3"] = np.asarray(p["bL3"], np.float32).reshape(-1, 1)
    return q


# ---------------------------------------------------------------------------
class G:
    """builder globals"""
    nc = None
    ll = None
    probe = None
    tc = None
    pools = None
    W = None
    cci = 0


def leaky(out_ap, in_ap, s_ap, t_ap, accum_out=None):
    nc = G.nc
    nc.scalar.activation(out_ap, in_ap, AF.Identity, bias=t_ap, scale=s_ap)
    nc.vector.scalar_tensor_tensor(
        out_ap, out_ap, 0.2, out_ap, ALU.mult, ALU.max, accum_out=accum_out
    )


def wchunks(name, C, O, tag=None, dual=False):
    """Load a [C, O] transposed-weight DRAM tensor into SBUF as contract
    chunks; returns list of (ap_fn) where chunk ci -> ap rows [0:nr], cols O."""
    nc, (pers, wpool, work, worka, small, psum, psum1, dram) = G.nc, G.pools
    nch = (C + 127) // 128
    t = wpool.tile([128, nch, O], F32, tag=tag or name)
    if nch == 1:
        nc.sync.dma_start(out=t[0:C, 0, 0:O], in_=G.W[name][:])
        if dual:
            assert C <= 64
            nc.sync.dma_start(out=t[64 : 64 + C, 0, 0:O], in_=G.W[name][:])
    else:
        nc.sync.dma_start(
            out=t[:], in_=G.W[name][:].rearrange("(c p) o -> p c o", p=128)
        )
    return [t[:, ci, :] for ci in range(nch)], t


def bn_coeffs(sums_ap, ssq_ap, g_ap, b_ap, inv_cnt, rows, uid):
    nc, (pers, wpool, work, worka, small, psum, psum1, dram) = G.nc, G.pools
    mean = small.tile([rows, 1], F32, tag="bnm")
    e2 = small.tile([rows, 1], F32, tag="bne")
    s = small.tile([rows, 1], F32, tag=f"bns{uid}")
    t = small.tile([rows, 1], F32, tag=f"bnt{uid}")
    nc.vector.tensor_scalar_mul(mean[:], sums_ap, float(inv_cnt))
    nc.vector.tensor_scalar_mul(e2[:], ssq_ap, float(inv_cnt))
    # e2 <- mean^2 - e2 ; e2 <- -e2 = var ; sd = sqrt(var+eps)
    nc.vector.scalar_tensor_tensor(e2[:], mean[:], mean[:], e2[:], ALU.mult, ALU.subtract)
    nc.vector.tensor_scalar_mul(e2[:], e2[:], -1.0)
    nc.vector.tensor_scalar_add(e2[:], e2[:], float(EPS))
    nc.scalar.activation(e2[:], e2[:], AF.Sqrt)
    nc.vector.reciprocal(s[:], e2[:])
    nc.vector.tensor_tensor(out=s[:], in0=s[:], in1=g_ap, op=ALU.mult)
    nc.vector.tensor_tensor(out=t[:], in0=mean[:], in1=s[:], op=ALU.mult)
    nc.vector.scalar_tensor_tensor(t[:], t[:], -1.0, b_ap, ALU.mult, ALU.add)
    return s, t


def allreduce_bn(stat, rows, nh, bn_i, inv_cnt):
    """AllReduce stat [rows, 2*nh]; returns (s_all, t_all) [rows, nh]."""
    nc, (pers, wpool, work, worka, small, psum, psum1, dram) = G.nc, G.pools
    G.cci += 1
    u = G.cci
    ci = dram.tile([rows, 2 * nh], F32, tag=f"cci{u}")
    co = dram.tile([rows, 2 * nh], F32, tag=f"cco{u}")
    nc.sync.dma_start(out=ci[:], in_=stat[0:rows, :])
    if os.environ.get("KERNEL_NO_CC", "") == "1":
        nc.sync.dma_start(out=co[:], in_=ci[:])
    else:
        nc.gpsimd.collective_compute(
            "AllReduce", ALU.add, replica_groups=[list(range(NCORES))],
            ins=[ci.opt()], outs=[co.opt()],
        )
    red = small.tile([rows, 2 * nh], F32, tag="ccred")
    nc.sync.dma_start(out=red[:], in_=co[:])
    g_sb = small.tile([rows, nh], F32, tag="ccg")
    b_sb = small.tile([rows, nh], F32, tag="ccb")
    nc.sync.dma_start(
        out=g_sb[:], in_=G.W[f"g{bn_i}"][:].rearrange("(h p) a -> p (h a)", p=rows)
    )
    nc.sync.dma_start(
        out=b_sb[:], in_=G.W[f"b{bn_i}"][:].rearrange("(h p) a -> p (h a)", p=rows)
    )
    s_all = small.tile([rows, nh], F32, tag=f"sall{u}")
    t_all = small.tile([rows, nh], F32, tag=f"tall{u}")
    for h in range(nh):
        s, t = bn_coeffs(
            red[:, 2 * h : 2 * h + 1], red[:, 2 * h + 1 : 2 * h + 2],
            g_sb[:, h : h + 1], b_sb[:, h : h + 1], inv_cnt, rows, "c",
        )
        nc.vector.tensor_copy(s_all[:, h : h + 1], s[:])
        nc.vector.tensor_copy(t_all[:, h : h + 1], t[:])
    return s_all, t_all


def stat_into(stat, col, su, sq, rlo, nr, first):
    nc = G.nc
    if first:
        nc.vector.tensor_copy(stat[rlo : rlo + nr, col : col + 1], su)
        nc.vector.tensor_copy(stat[rlo : rlo + nr, col + 1 : col + 2], sq)
    else:
        nc.vector.tensor_tensor(
            out=stat[rlo : rlo + nr, col : col + 1],
            in0=stat[rlo : rlo + nr, col : col + 1], in1=su, op=ALU.add)
        nc.vector.tensor_tensor(
            out=stat[rlo : rlo + nr, col + 1 : col + 2],
            in0=stat[rlo : rlo + nr, col + 1 : col + 2], in1=sq, op=ALU.add)


def pack_fold(stat, small):
    """packed mode: add rows 64-127 into rows 0-63 (both stat cols)."""
    nc = G.nc
    sh = small.tile([128, 2], F32, tag="pfold")
    nc.sync.dma_start(out=sh[0:64, :], in_=stat[64:128, 0:2])
    nc.vector.tensor_tensor(
        out=stat[0:64, 0:2], in0=stat[0:64, 0:2], in1=sh[0:64, :], op=ALU.add
    )


def replicate_col(dst, src_col, small_unused):
    """packed: replicate a [64,1] col (rows 0-63) into [128,1] dst rows."""
    nc = G.nc
    nc.sync.dma_start(out=dst[0:64, :], in_=src_col)
    nc.sync.dma_start(out=dst[64:128, :], in_=src_col)


# ---------------------------------------------------------------------------
# activations: acts[s] = list of tiles (one per 128-channel block), each
# [nr, N] at partition base 0.  L1/L2: one [64,N] tile; L3: one [128,N];
# L4: two [128,N].
def emit_edge_layer(li, acts, ones):
    nc, (pers, wpool, work, worka, small, psum, psum1, dram) = G.nc, G.pools
    C, O = EDGE[li]
    bn_i = [1, 4, 7, 10][li]
    packed = O == 64
    nh = max(1, O // 128)
    nr_in = [C, 64, 64, 128][li]
    wl_ch, _ = wchunks(f"wlT{li}", C, O, tag="wlT")
    wd_ch, _ = wchunks(f"wdT{li}", C, O, tag="wdT")

    # ---------------- pd + topk per sample -> wrap tiles -------------------
    wraps = []
    A_s = {}
    Bc_s = {}
    for s in range(BL):
        xt = acts[s]  # list of input tiles
        xsq = work.tile([128, N], F32, tag="scratch")
        bt = xt[0]  # C <= 128 always: single input tile
        nc.vector.tensor_tensor(out=xsq[0:C, :], in0=bt[0:C, :], in1=bt[0:C, :],
                                op=ALU.mult)
        nxx = worka.tile([1, N], F32, tag="nxx")  # -xx/2 (for the aug matmul row)
        nxf = worka.tile([1, N], F32, tag="nxf")  # -xx   (for the ACT bias)
        for ch in range(2):
            pxx = psum1.tile([1, 512], F32, tag="ps1")
            nc.tensor.matmul(pxx[:], ones[0:C, 0:1],
                             xsq[0:C, 512 * ch : 512 * ch + 512],
                             start=True, stop=True)
            nc.vector.tensor_scalar_mul(nxx[:, 512 * ch : 512 * ch + 512], pxx[:], -0.5)
            nc.vector.tensor_scalar_mul(nxf[:, 512 * ch : 512 * ch + 512], pxx[:], -1.0)
        nxd = dram.tile([N], F32, tag="nxd")
        nc.sync.dma_start(out=nxd[:], in_=nxf[:])
        nxT = small.tile([128, 8], F32, tag="nxT")
        nc.sync.dma_start(out=nxT[:], in_=nxd[:].rearrange("(b p) -> p b", p=128))

        jline = dram.tile([N * 20], U16, tag="jline")
        for rb in range(8):
            pd = work.tile([128, N], F32, tag="pd")
            for ch in range(2):
                pdp = psum.tile([128, 512], F32, tag="ps")
                nc.tensor.matmul(pdp[:], bt[0:C, 128 * rb : 128 * rb + 128],
                                 bt[0:C, 512 * ch : 512 * ch + 512],
                                 start=True, stop=False)
                nc.tensor.matmul(pdp[:], ones[0:1, 0:128],
                                 nxx[:, 512 * ch : 512 * ch + 512],
                                 start=False, stop=True)
                nc.scalar.activation(pd[:, 512 * ch : 512 * ch + 512], pdp[:],
                                     AF.Identity, bias=nxT[:, rb : rb + 1], scale=2.0)
            v24 = small.tile([128, 24], F32, tag="v24")
            i24 = small.tile([128, 24], U16, tag="i24")
            for r in range(3):
                nc.vector.max(v24[:, 8 * r : 8 * r + 8], pd[:])
                nc.vector.max_index(i24[:, 8 * r : 8 * r + 8],
                                    v24[:, 8 * r : 8 * r + 8], pd[:])
                if r < 2:
                    nc.vector.match_replace(pd[:], v24[:, 8 * r : 8 * r + 8], pd[:], NEG)
            nc.sync.dma_start(
                out=jline[2560 * rb : 2560 * rb + 2560].rearrange("(p k) -> p k", p=128),
                in_=i24[:, 0:20])
        wr = worka.tile([128, 1280], U16, tag=f"wrap{s}")
        nc.sync.dma_start(out=wr[0:16, :], in_=jline[:].rearrange("(s p) -> p s", p=16))
        for g in range(1, 8):
            nc.sync.dma_start(out=wr[16 * g : 16 * g + 16, :], in_=wr[0:16, :])
        wraps.append(wr)

        # ---------------- A, Bc per sample (base 0) ------------------------
        A_s[s] = []
        Bc_s[s] = []
        for h in range(nh):
            ow = min(128, O)
            At = work.tile([128, N], F32, tag="A")
            Bt = work.tile([128, N], F32, tag=f"Bc{h}")
            for ch in range(2):
                pA = psum.tile([128, 512], F32, tag="ps")
                pB = psum.tile([128, 512], F32, tag="ps")
                nc.tensor.matmul(pA[0:ow, :], wl_ch[0][0:C, 128 * h : 128 * h + ow],
                                 bt[0:C, 512 * ch : 512 * ch + 512],
                                 start=True, stop=True)
                nc.tensor.matmul(pB[0:ow, :], wd_ch[0][0:C, 128 * h : 128 * h + ow],
                                 bt[0:C, 512 * ch : 512 * ch + 512],
                                 start=True, stop=True)
                nc.scalar.copy(At[0:ow, 512 * ch : 512 * ch + 512], pA[0:ow, :])
                nc.scalar.copy(Bt[0:ow, 512 * ch : 512 * ch + 512], pB[0:ow, :])
            A_s[s].append(At)
            Bc_s[s].append(Bt)

    # ---------------- gather blocks ----------------------------------------
    stat = small.tile([128, 2 * nh], F32, tag="estat")
    out_m = []

    def gather_block(Ag, Bg, wuse, h, first, gi_tag):
        Gm = work.tile([128, N], F32, tag=f"Gm{gi_tag}")
        G1 = work.tile([128, N], F32, tag="G1")
        ssqp = small.tile([128, 16], F32, tag="ssqp")
        NCH = 16
        npts = N // NCH
        for c in range(NCH):
            gth = work.tile([128, npts * 20], F32, tag="gth")
            _gi = nc.gpsimd.ap_gather(
                out_ap=gth[:].rearrange("p (n k) -> p n k", k=1),
                in_ap=Ag[:].rearrange("p (n k) -> p n k", k=1),
                idxs_ap=wuse[:, (npts * 20 // 16) * c : (npts * 20 // 16) * (c + 1)].bitcast(I16),
                channels=128, num_elems=N, d=1, num_idxs=npts * 20)
            add_dep_helper(G.ll.ins, _gi.ins, sync=False, reason="lib before gather")
            g3 = gth[:].rearrange("p (n k) -> p n k", k=20)
            nc.vector.tensor_reduce(Gm[:, npts * c : npts * (c + 1)], g3, AX.X, ALU.max)
            nc.vector.tensor_reduce(G1[:, npts * c : npts * (c + 1)], g3, AX.X, ALU.add)
            nc.scalar.activation(gth[:], gth[:], AF.Square, accum_out=ssqp[:, c : c + 1])

        su = small.tile([128, 1], F32, tag="su")
        sq = small.tile([128, 1], F32, tag="sq")
        t1 = small.tile([128, 1], F32, tag="t1")
        t2 = small.tile([128, 1], F32, tag="t2")
        nc.vector.tensor_reduce(t1[:], G1[:], AX.X, ALU.add)
        nc.vector.tensor_reduce(t2[:], Bg[:], AX.X, ALU.add)
        nc.vector.scalar_tensor_tensor(su[:], t2[:], float(K), t1[:], ALU.mult, ALU.add)
        nc.vector.tensor_reduce(sq[:], ssqp[:], AX.X, ALU.add)
        nc.vector.tensor_tensor(out=G1[:], in0=Bg[:], in1=G1[:], op=ALU.mult)
        nc.vector.tensor_reduce(t1[:], G1[:], AX.X, ALU.add)
        nc.vector.scalar_tensor_tensor(sq[:], t1[:], 2.0, sq[:], ALU.mult, ALU.add)
        nc.vector.tensor_tensor(out=G1[:], in0=Bg[:], in1=Bg[:], op=ALU.mult)
        nc.vector.tensor_reduce(t1[:], G1[:], AX.X, ALU.add)
        nc.vector.scalar_tensor_tensor(sq[:], t1[:], float(K), sq[:], ALU.mult, ALU.add)
        stat_into(stat, 2 * h, su[:], sq[:], 0, 128, first=first)
        out_m.append((Gm, Bg))

    if packed:
        Ag = work.tile([128, N], F32, tag="pd")
        Bg = work.tile([128, N], F32, tag="Bc1")
        for s in range(BL):
            nc.sync.dma_start(out=Ag[64 * s : 64 * s + 64, :], in_=A_s[s][0][0:64, :])
            nc.sync.dma_start(out=Bg[64 * s : 64 * s + 64, :], in_=Bc_s[s][0][0:64, :])
        wuse = work.tile([128, 1280], U16, tag="scratch")
        for g in range(4):
            nc.sync.dma_start(out=wuse[16 * g : 16 * g + 16, :], in_=wraps[0][0:16, :])
        for g in range(4, 8):
            nc.sync.dma_start(out=wuse[16 * g : 16 * g + 16, :], in_=wraps[1][0:16, :])
        gather_block(Ag, Bg, wuse, 0, True, 0)
    else:
        for s in range(BL):
            for h in range(nh):
                gather_block(A_s[s][h], Bc_s[s][h], wraps[s], h, s == 0, h)

    if packed:
        pack_fold(stat, small)
    rows_bn = 64 if packed else 128
    s_a, t_a = allreduce_bn(stat, rows_bn, nh, bn_i, 1.0 / CNT_EDGE)

    # ---------------- apply ------------------------------------------------
    xo_tags = [["xo0a", "xo0b"], ["xo1a", "xo1b"], ["xo0a", "xo0b"],
               ["xo1a", "xo1b", "xo2a", "xo2b"]][li]
    newa = [[] for _ in range(BL)]
    if packed:
        Gm, Bg = out_m[0]
        sr = small.tile([128, 1], F32, tag="srep")
        tr = small.tile([128, 1], F32, tag="trep")
        replicate_col(sr, s_a[0:64, 0:1], small)
        replicate_col(tr, t_a[0:64, 0:1], small)
        nc.vector.tensor_tensor(out=Gm[:], in0=Gm[:], in1=Bg[:], op=ALU.add)
        xp = work.tile([128, N], F32, tag="scratch")
        leaky(xp[:], Gm[:], sr[:], tr[:])
        for s in range(BL):
            xo = pers.tile([64, N], F32, tag=xo_tags[s])
            nc.sync.dma_start(out=xo[:], in_=xp[64 * s : 64 * s + 64, :])
            newa[s] = [xo]
    else:
        bi = 0
        for s in range(BL):
            for h in range(nh):
                Gm, Bg = out_m[bi]
                bi += 1
                xo = pers.tile([128, N], F32, tag=xo_tags[s * nh + h])
                nc.vector.tensor_tensor(out=Gm[:], in0=Gm[:], in1=Bg[:], op=ALU.add)
                leaky(xo[:], Gm[:], s_a[:, h : h + 1], t_a[:, h : h + 1])
                newa[s].append(xo)
    return newa


# ---------------------------------------------------------------------------
def emit_eblock(li, acts):
    """edited_by_lg; acts per-sample tile lists. Returns e in same layout."""
    nc, (pers, wpool, work, worka, small, psum, psum1, dram) = G.nc, G.pools
    O = EDGE[li][1]
    bn_a = [2, 5, 8, 11][li]
    bn_b = [3, 6, 9, 12][li]
    nh = max(1, O // 128)
    nr = min(128, O)
    wal_ch, _ = wchunks(f"walT{li}", O, O, tag="walT")
    war_ch, _ = wchunks(f"warT{li}", O, O, tag="warT")
    wb_ch, _ = wchunks(f"wbT{li}", O, O, tag="wbT")

    globs = {}
    for s in range(BL):
        gl = small.tile([128, nh], F32, tag=f"glob{s}")
        for hh, bt in enumerate(acts[s]):
            nc.vector.tensor_reduce(gl[0:nr, hh : hh + 1], bt[0:nr, :], AX.X, ALU.max)
        globs[s] = gl

    def conv_to(wch, src_tiles, h, dst, pv_dst=None, gl=None):
        """dst[0:nr, :] = sum_ci wch[ci][:, 128h:+nr].T @ src_tiles[ci]"""
        for ch in range(2):
            pp = psum.tile([128, 512], F32, tag="ps")
            for ci, bt in enumerate(src_tiles):
                cr = min(128, O - 128 * ci)
                nc.tensor.matmul(pp[0:nr, :], wch[ci][0:cr, 128 * h : 128 * h + nr],
                                 bt[0:cr, 512 * ch : 512 * ch + 512],
                                 start=(ci == 0), stop=(ci == len(src_tiles) - 1))
            nc.scalar.copy(dst[0:nr, 512 * ch : 512 * ch + 512], pp[0:nr, :])
        if pv_dst is not None:
            pv = psum1.tile([128, 1], F32, tag="ps1")
            for ci in range(len(src_tiles)):
                cr = min(128, O - 128 * ci)
                nc.tensor.matmul(pv[0:nr, :], war_ch[ci][0:cr, 128 * h : 128 * h + nr],
                                 gl[0:cr, ci : ci + 1],
                                 start=(ci == 0), stop=(ci == len(src_tiles) - 1))
            nc.scalar.copy(pv_dst[0:nr, :], pv[0:nr, :])

    # conv1: u, v ; stats of u+v over (B, N)
    stat = small.tile([128, 2 * nh], F32, tag="lstat")
    ub = {}
    for s in range(BL):
        ub[s] = []
        for h in range(nh):
            ut = work.tile([128, N], F32, tag=f"eu{h}")
            vt = small.tile([128, 1], F32, tag=f"ev{s}{h}")
            conv_to(wal_ch, acts[s], h, ut, pv_dst=vt, gl=globs[s])
            ub[s].append((ut, vt))
            su = small.tile([128, 1], F32, tag="su")
            sq = small.tile([128, 1], F32, tag="sq")
            t2 = small.tile([128, 1], F32, tag="t2")
            sc = work.tile([128, N], F32, tag="scratch")
            nc.vector.tensor_reduce(su[0:nr, :], ut[0:nr, :], AX.X, ALU.add)
            nc.scalar.activation(sc[0:nr, :], ut[0:nr, :], AF.Square,
                                 accum_out=sq[0:nr, :])
            nc.vector.tensor_tensor(out=t2[0:nr, :], in0=vt[0:nr, :], in1=su[0:nr, :],
                                    op=ALU.mult)
            nc.vector.scalar_tensor_tensor(sq[0:nr, :], t2[0:nr, :], 2.0, sq[0:nr, :],
                                           ALU.mult, ALU.add)
            nc.vector.tensor_tensor(out=t2[0:nr, :], in0=vt[0:nr, :], in1=vt[0:nr, :],
                                    op=ALU.mult)
            nc.vector.scalar_tensor_tensor(sq[0:nr, :], t2[0:nr, :], float(N),
                                           sq[0:nr, :], ALU.mult, ALU.add)
            nc.vector.scalar_tensor_tensor(su[0:nr, :], vt[0:nr, :], float(N),
                                           su[0:nr, :], ALU.mult, ALU.add)
            stat_into(stat, 2 * h, su[0:nr, :], sq[0:nr, :], 0, nr, first=(s == 0))
    s1, t1b = allreduce_bn(stat, nr, nh, bn_a, 1.0 / CNT_CONV)

    # LG = leaky(s1*(u+v)+t1) ; d = LG - x
    d_t = {}
    for s in range(BL):
        d_t[s] = []
        for h in range(nh):
            ut, vt = ub[s][h]
            tb = small.tile([128, 1], F32, tag="tb1")
            nc.vector.tensor_tensor(out=tb[0:nr, :], in0=vt[0:nr, :],
                                    in1=s1[0:nr, h : h + 1], op=ALU.mult)
            nc.vector.tensor_tensor(out=tb[0:nr, :], in0=tb[0:nr, :],
                                    in1=t1b[0:nr, h : h + 1], op=ALU.add)
            dt = work.tile([128, N], F32, tag=f"ed{h}")
            leaky(dt[0:nr, :], ut[0:nr, :], s1[0:nr, h : h + 1], tb[0:nr, :])
            nc.vector.tensor_tensor(out=dt[0:nr, :], in0=dt[0:nr, :],
                                    in1=acts[s][h][0:nr, :], op=ALU.subtract)
            d_t[s].append(dt)

    # conv2 w = Wb @ d ; stats ; e = x + leaky(s2*w + t2)
    stat2 = small.tile([128, 2 * nh], F32, tag="lstat2")
    w_t = {}
    for s in range(BL):
        w_t[s] = []
        for h in range(nh):
            wt = work.tile([128, N], F32, tag=f"eu{h}")
            conv_to(wb_ch, d_t[s], h, wt)
            w_t[s].append(wt)
            su = small.tile([128, 1], F32, tag="su")
            sq = small.tile([128, 1], F32, tag="sq")
            sc = work.tile([128, N], F32, tag="scratch")
            nc.vector.tensor_reduce(su[0:nr, :], wt[0:nr, :], AX.X, ALU.add)
            nc.scalar.activation(sc[0:nr, :], wt[0:nr, :], AF.Square,
                                 accum_out=sq[0:nr, :])
            stat_into(stat2, 2 * h, su[0:nr, :], sq[0:nr, :], 0, nr, first=(s == 0))
    s2, t2b = allreduce_bn(stat2, nr, nh, bn_b, 1.0 / CNT_CONV)

    e_out = [[] for _ in range(BL)]
    for s in range(BL):
        for h in range(nh):
            et = pers.tile([nr, N], F32, tag=f"e{li}_{s}_{h}")
            leaky(et[0:nr, :], w_t[s][h][0:nr, :], s2[0:nr, h : h + 1],
                  t2b[0:nr, h : h + 1])
            nc.vector.tensor_tensor(out=et[0:nr, :], in0=et[0:nr, :],
                                    in1=acts[s][h][0:nr, :], op=ALU.add)
            e_out[s].append(et)
    return e_out


# ---------------------------------------------------------------------------
def emit_head_and_fc(e_out, out_t):
    nc, (pers, wpool, work, worka, small, psum, psum1, dram) = G.nc, G.pools
    e1, e2, e3, e4 = e_out

    stages = []
    for s in range(BL):
        st = pers.tile([128, N], F32, tag=["xo0a", "xo0b"][s])
        nc.sync.dma_start(out=st[0:64, :], in_=e1[s][0][0:64, :])
        nc.sync.dma_start(out=st[64:128, :], in_=e2[s][0][0:64, :])
        stages.append(st)

    def chunks_for(s):
        return [
            (stages[s], 0, 128, 0),
            (e3[s][0], 0, 128, 128),
            (e4[s][0], 0, 128, 256),
            (e4[s][1], 0, 128, 384),
        ]

    def w13_block(ob):
        """stream w13T columns for out-block ob: [128, 4, 128] tile."""
        t = wpool.tile([128, 4, 128], F32, tag="w13b")
        nc.sync.dma_start(
            out=t[:],
            in_=G.W["w13T"][:, 128 * ob : 128 * ob + 128].rearrange(
                "(c p) o -> p c o", p=128))
        return t

    def mm13(pp, wt, ch, fc):
        for ci, (et, rlo, nr, wlo) in enumerate(ch):
            q, r0 = wlo // 128, wlo % 128
            nc.tensor.matmul(
                pp[:], wt[r0 : r0 + nr, q, :],
                et[rlo : rlo + nr, 512 * fc : 512 * fc + 512],
                start=(ci == 0), stop=(ci == len(ch) - 1))

    # pass 1: stats
    stat = small.tile([128, 16], F32, tag="fstat")
    for ob in range(8):
        wt = w13_block(ob)
        for s in range(BL):
            ch = chunks_for(s)
            for fc in range(2):
                pp = psum.tile([128, 512], F32, tag="ps")
                mm13(pp, wt, ch, fc)
                su = small.tile([128, 1], F32, tag="su")
                sq = small.tile([128, 1], F32, tag="sq")
                sc = work.tile([128, 512], F32, tag="scratch")
                nc.vector.tensor_reduce(su[:], pp[:], AX.X, ALU.add)
                nc.scalar.activation(sc[:], pp[:], AF.Square, accum_out=sq[:])
                stat_into(stat, 2 * ob, su[:], sq[:], 0, 128, first=(s == 0 and fc == 0))
    s13, t13 = allreduce_bn(stat, 128, 8, 13, 1.0 / CNT_CONV)

    # pass 2: recompute, BN+leaky, pool
    featT = pers.tile([128, 16, BL], F32, tag="featT")
    for ob in range(8):
        wt = w13_block(ob)
        for s in range(BL):
            ch = chunks_for(s)
            hmax = small.tile([128, 2], F32, tag="fhm")
            hsum = small.tile([128, 2], F32, tag="fhs")
            for fc in range(2):
                pp = psum.tile([128, 512], F32, tag="ps")
                mm13(pp, wt, ch, fc)
                htile = work.tile([128, 512], F32, tag="scratch")
                leaky(htile[:], pp[:], s13[:, ob : ob + 1], t13[:, ob : ob + 1],
                      accum_out=hsum[:, fc : fc + 1])
                nc.vector.tensor_reduce(hmax[:, fc : fc + 1], htile[:], AX.X, ALU.max)
            nc.vector.tensor_tensor(out=hmax[:, 0:1], in0=hmax[:, 0:1],
                                    in1=hmax[:, 1:2], op=ALU.max)
            nc.vector.tensor_tensor(out=hsum[:, 0:1], in0=hsum[:, 0:1],
                                    in1=hsum[:, 1:2], op=ALU.add)
            nc.vector.tensor_copy(featT[:, ob, s : s + 1], hmax[:, 0:1])
            nc.vector.tensor_scalar_mul(featT[:, 8 + ob, s : s + 1], hsum[:, 0:1], 1.0 / N)

    # ---------------- FC layers -------------------------------------------
    def fc_bn_layer(rhs_blocks, lname, fout, bn_i, bias_d, uid):
        nb = fout // 128
        ncin = len(rhs_blocks)
        stat = small.tile([128, 2 * nb], F32, tag=f"fcst{uid}")
        ys = []
        for ob in range(nb):
            pp = psum1.tile([128, BL], F32, tag="ps1")
            for gi in range((ncin + 3) // 4):
                cis = list(range(4 * gi, min(4 * gi + 4, ncin)))
                lw = wpool.tile([128, len(cis), 128], F32, tag="w13b")
                nc.sync.dma_start(
                    out=lw[:],
                    in_=G.W[lname][128 * cis[0] : 128 * (cis[-1] + 1),
                                   128 * ob : 128 * ob + 128].rearrange(
                        "(c p) o -> p c o", p=128))
                for j, ci in enumerate(cis):
                    nc.tensor.matmul(pp[:], lw[:, j, :], rhs_blocks[ci],
                                     start=(ci == 0), stop=(ci == ncin - 1))
            y = small.tile([128, BL], F32, tag=f"fcy{uid}_{ob}")
            if bias_d is not None:
                bb = small.tile([128, 1], F32, tag="fcb")
                nc.sync.dma_start(
                    out=bb[:],
                    in_=bias_d[:].rearrange("(h p) a -> p (h a)", p=128)[:, ob : ob + 1])
                nc.scalar.activation(y[:], pp[:], AF.Identity, bias=bb[:], scale=1.0)
            else:
                nc.scalar.copy(y[:], pp[:])
            ys.append(y)
            su = small.tile([128, 1], F32, tag="su")
            sq = small.tile([128, 1], F32, tag="sq")
            sc = small.tile([128, BL], F32, tag="fcsc")
            nc.vector.tensor_reduce(su[:], y[:], AX.X, ALU.add)
            nc.scalar.activation(sc[:], y[:], AF.Square, accum_out=sq[:])
            stat_into(stat, 2 * ob, su[:], sq[:], 0, 128, first=True)
        s_a, t_a = allreduce_bn(stat, 128, nb, bn_i, 1.0 / B)
        outs = []
        for ob in range(nb):
            o = small.tile([128, BL], F32, tag=f"fco{uid}_{ob}")
            leaky(o[:], ys[ob][:], s_a[:, ob : ob + 1], t_a[:, ob : ob + 1])
            outs.append(o[:])
        return outs

    feat_blocks = [featT[:, c, :] for c in range(16)]
    h1 = fc_bn_layer(feat_blocks, "l1T", 512, 14, None, "1")
    h2 = fc_bn_layer(h1, "l2T", 256, 15, G.W["bL2"], "2")

    l3w = wpool.tile([128, 2, 40], F32, tag="l3w")
    nc.sync.dma_start(out=l3w[:], in_=G.W["l3T"][:].rearrange("(c p) o -> p c o", p=128))
    pp = psum1.tile([40, BL], F32, tag="ps1")
    for ci in range(2):
        nc.tensor.matmul(pp[:], l3w[:, ci, :], h2[ci], start=(ci == 0), stop=(ci == 1))
    b3 = small.tile([40, 1], F32, tag="b3")
    nc.sync.dma_start(out=b3[:], in_=G.W["bL3"][:])
    o3 = small.tile([40, BL], F32, tag="o3")
    nc.scalar.activation(o3[:], pp[:], AF.Identity, bias=b3[:], scale=1.0)
    nc.sync.dma_start(out=out_t[:].rearrange("b f -> f b"), in_=o3[:])


# ---------------------------------------------------------------------------
def build():
    nc = bacc.Bacc("TRN2", target_bir_lowering=False, debug=False,
                   num_devices=(1 if os.environ.get("KERNEL_NO_CC", "") == "1" else NCORES))
    G.nc = nc
    x_in = nc.dram_tensor("x", [BL, C0, N], F32, kind="ExternalInput")
    W = {}
    for li, (C, O) in enumerate(EDGE):
        W[f"wlT{li}"] = nc.dram_tensor(f"wlT{li}", [C, O], F32, kind="ExternalInput")
        W[f"wdT{li}"] = nc.dram_tensor(f"wdT{li}", [C, O], F32, kind="ExternalInput")
        W[f"walT{li}"] = nc.dram_tensor(f"walT{li}", [O, O], F32, kind="ExternalInput")
        W[f"warT{li}"] = nc.dram_tensor(f"warT{li}", [O, O], F32, kind="ExternalInput")
        W[f"wbT{li}"] = nc.dram_tensor(f"wbT{li}", [O, O], F32, kind="ExternalInput")
    W["w13T"] = nc.dram_tensor("w13T", [512, 1024], F32, kind="ExternalInput")
    W["l1T"] = nc.dram_tensor("l1T", [2048, 512], F32, kind="ExternalInput")
    W["l2T"] = nc.dram_tensor("l2T", [512, 256], F32, kind="ExternalInput")
    W["l3T"] = nc.dram_tensor("l3T", [256, 40], F32, kind="ExternalInput")
    chs = [64, 64, 64, 64, 64, 64, 128, 128, 128, 256, 256, 256, 1024, 512, 256]
    for i in range(1, 16):
        W[f"g{i}"] = nc.dram_tensor(f"g{i}", [chs[i - 1], 1], F32, kind="ExternalInput")
        W[f"b{i}"] = nc.dram_tensor(f"b{i}", [chs[i - 1], 1], F32, kind="ExternalInput")
    W["bL2"] = nc.dram_tensor("bL2", [256, 1], F32, kind="ExternalInput")
    W["bL3"] = nc.dram_tensor("bL3", [40, 1], F32, kind="ExternalInput")
    G.W = W
    out_t = nc.dram_tensor("out", [BL, 40], F32, kind="ExternalOutput")

    with tile.TileContext(nc) as tc:
        G.tc = tc
        import contextlib

        with contextlib.ExitStack() as ctx:
            pers = ctx.enter_context(tc.tile_pool(name="pers", bufs=1))
            wpool = ctx.enter_context(tc.tile_pool(name="wpool", bufs=1))
            work = ctx.enter_context(tc.tile_pool(name="work", bufs=2))
            worka = ctx.enter_context(tc.tile_pool(name="worka", bufs=1))
            small = ctx.enter_context(tc.tile_pool(name="small", bufs=1))
            psum = ctx.enter_context(tc.tile_pool(name="psum", bufs=3, space="PSUM"))
            psum1 = ctx.enter_context(tc.tile_pool(name="psum1", bufs=2, space="PSUM"))
            dram = ctx.enter_context(tc.tile_pool(name="dram", bufs=2, space="DRAM"))
            G.pools = (pers, wpool, work, worka, small, psum, psum1, dram)

            G.ll = nc.gpsimd.load_library(library_config.ap_gather)
            ones = pers.tile([128, 128], F32, tag="ones")
            nc.vector.memset(ones[:], 1.0)
            x0s = []
            for s in range(BL):
                t = pers.tile([C0, N], F32, tag=["xo2a", "xo2b"][s])
                nc.sync.dma_start(out=t[:], in_=x_in[s])
                x0s.append(t)

            kstop = int(os.environ.get("KSTOP", "10"))
            acts = [[x0s[0]], [x0s[1]]]
            e_out = []
            done = False
            for li in range(4):
                if kstop <= 2 * li:
                    done = True
                    break
                acts = emit_edge_layer(li, acts, ones)
                if kstop <= 2 * li + 1:
                    done = True
                    break
                e_out.append(emit_eblock(li, acts))
            if not done and kstop >= 9:
                emit_head_and_fc(e_out, out_t)
            else:
                z = pers.tile([40, BL], F32, tag="zout")
                nc.vector.memset(z[:], 0.0)
                src_t = G.probe if G.probe is not None else acts[0][0]
                nc.vector.tensor_copy(z[:], src_t[0:min(40, src_t.shape[0]), 0:BL].bitcast(F32)[0:40, 0:BL])
                nc.sync.dma_start(out=out_t[:].rearrange("b f -> f b"), in_=z[:])

    nc.compile()
    return nc


def kernel(x, params):
    x = np.asarray(x, np.float32)
    q = prep_params(params)
    if "nc" not in _CACHE:
        _CACHE["nc"] = build()
    nc = _CACHE["nc"]
    in_maps = []
    for c in range(NCORES):
        m = {"x": np.ascontiguousarray(x[BL * c : BL * c + BL])}
        m.update(q)
        in_maps.append(m)
    r = run_bass_kernel_spmd(nc, in_maps, list(range(NCORES)))
    return np.concatenate(
        [r.results[c]["out"] for c in range(NCORES)], axis=0
    ).astype(np.float32)
